# revision 2
# baseline (speedup 1.0000x reference)
"""Trainium2 Bass kernel for nn_AttentionBlock: 8-core data-parallel over batch.

Reference computation (per batch b):
  cx = X[b] @ Wx^T               [K,R]   (K=49 regions, R=49, H=1024)
  ch = h_t[b] @ Wh^T             [T,R]   (T=128)
  z[t,k] = sum_r Wa[r] * tanh(cx[k,r] + ch[t,r])
  alpha = softmax_k(z)           [T,K]
  out[b] = alpha @ X[b]          [T,H]

Design (measured 66064 ns vs the 148846 ns v3 baseline):
  - host pre-packs the DMA-heavy layouts:
    * hxT[pair, p, j, :]: bf16 h^T (cols 0:256, u=bb*128+t) and X^T (cols
      256:354, q=bb*49+k) tiles already in matmul-rhs layout -> one large-
      granule DMA per pair, no device transposes or casts at all
    * X padded to 64 rows bf16 (out-matmul rhs; bb=1 block at partition
      base 64, which is a legal engine base)
  - inputs fed bf16, output written bf16 and upcast on host: HBM traffic
    ~12.5MB/core vs 20.4MB all-f32
  - ch/cx via DOUBLED stationaries [Wh|0|Wh], [Wx|0|Wx] built on device:
    cc[113, 0:256]=ch at rows 0:49 AND 64:113, cc[113, 256:354]=cx same;
    gap rows 49:64 are zeros (zero stationary cols)
  - rank-6 tanh-product fit tanh(a+b) ~= sum_m c_m T(g_m a+d_m)T(g'_m b+d'_m)
    (m-pairs packed at partition blocks 0:49 / 64:113, 3 ACT ops per side
    with per-partition scale/bias); z as 3 PSUM-accumulated matmuls per batch
  - z computed TRANSPOSED zT[k, (bb,t)]: softmax needs no PE transpose of
    alpha; no max-subtraction (|z| <= ~6 analytically); denominator via a
    1-column ones-matmul; 1/den folded into the PSUM->SBUF output drain as a
    free-axis-broadcast multiply on DVE
  - 6-deep software pipeline; emission order per iteration only references
    work >= 1 iteration old per engine, so no engine queue head-of-line
    blocks on same-iteration work from another engine
  - PE runs at its high P-state once streams are dense (ramps after ~3us of
    continuous execution; sparse streams run at half clock)
  - pitfalls baked in (each cost a debugging session):
    * engine ops need partition bases that are multiples of 32
    * gpsimd (Pool) cannot touch PSUM
    * the gpsimd memset in S_repl looks redundant but removing it makes the
      NEFF fail at runtime (scheduling artifact); same for odd-inner-dim
      ExternalInputs like [128, 15] f32 -- keep inputs 16/128-padded
    * concurrent XBAR dma_start_transpose ops on both HWDGE rings corrupt
      each other (not used in this version, but measured)
  - kernel() retries run_bass_kernel_spmd up to 3x: the axon device path
    intermittently fails with INTERNAL errors unrelated to the kernel
"""

import sys

sys.path.insert(0, "/opt/trn_rl_repo")

import numpy as np

import concourse.bass as bass
import concourse.bacc as bacc
import concourse.tile as tile
from concourse import mybir
from concourse.bass_utils import run_bass_kernel_spmd
from concourse.masks import make_identity

B, T, K, H = 128, 128, 49, 1024
R = 49
NCORES = 8
BL = B // NCORES  # batches per core
NP = BL // 2  # pairs per core
HT = H // 128  # h tiles
KP = 64  # padded K rows for X (host-side zero pad)
PB = 64  # partition base of the second (m, r) block (must be mult of 32)
PT = PB + R  # 113 partitions used
F32 = mybir.dt.float32
BF16 = mybir.dt.bfloat16

# rank-6 tanh-product fit (LAM=0.03 gauss-weighted, sigma=0.64, A=3.2):
# tanh(a+b) ~= sum_m FC[m] * tanh(FG[m]*a + FD[m]) * tanh(FGP[m]*b + FDP[m])
FG = [0.7368, 2.3523, 1.1871, 2.3100, 0.4495, 1.3332]
FD = [0.0554, 0.1456, -0.8720, 0.1260, -0.3369, -2.8210]
FGP = [-1.3332, -0.4495, 2.3100, 1.1871, 2.3523, 0.7368]
FDP = [-2.8210, -0.3369, -0.1260, 0.8720, -0.1456, -0.0554]
FC = [-1.0581, 1.7567, -0.9840, 0.9840, 1.7567, -1.0581]
NG = 3  # number of (m-pair) partition groups

_CACHE = {}


def _ap(base, off, dims):
    """Custom access pattern on the tensor underlying `base` (an AP)."""
    return bass.AP(tensor=base.tensor, offset=base.offset + off, ap=dims)


def build():
    nc = bacc.Bacc("TRN2", target_bir_lowering=False, debug=False, num_devices=NCORES)

    X_d = nc.dram_tensor("X", [BL, KP, H], BF16, kind="ExternalInput").ap()
    # host-packed transposed rhs tiles: hxT[pair, p, j, 0:256] = h^T block
    # (u = bb*128+t), cols 256:354 = X^T block (q = bb*49+k)
    hx_d = nc.dram_tensor("hxT", [NP, 128, HT, 354], BF16, kind="ExternalInput").ap()
    Wx_d = nc.dram_tensor("Wx", [R, H], F32, kind="ExternalInput").ap()
    Wh_d = nc.dram_tensor("Wh", [R, H], F32, kind="ExternalInput").ap()
    Wa_d = nc.dram_tensor("Wa", [1, R], F32, kind="ExternalInput").ap()
    prm_d = nc.dram_tensor("prm", [128, 16], F32, kind="ExternalInput").ap()
    out_d = nc.dram_tensor("out", [BL, T, H], BF16, kind="ExternalOutput").ap()

    with tile.TileContext(nc) as tc:
        with (
            tc.tile_pool(name="consts", bufs=1) as consts,
            tc.tile_pool(name="hxTp", bufs=3) as hxT_pool,
            tc.tile_pool(name="xbp", bufs=5) as xbp,
            tc.tile_pool(name="ccrp", bufs=3) as ccrp,
            tc.tile_pool(name="SAp", bufs=3) as SAp,
            tc.tile_pool(name="sbtp", bufs=2) as sbtp,
            tc.tile_pool(name="sbfp", bufs=3) as sbfp,
            tc.tile_pool(name="ezp", bufs=3) as ezp,
            tc.tile_pool(name="rdnp", bufs=3) as rdnp,
            tc.tile_pool(name="osbp", bufs=3) as osbp,
            tc.tile_pool(name="ptp", bufs=1, space="PSUM") as ptp,
            tc.tile_pool(name="pcc", bufs=2, space="PSUM") as pcc,
            tc.tile_pool(name="psZ", bufs=2, space="PSUM") as psZ,
            tc.tile_pool(name="psO", bufs=3, space="PSUM") as psO,
        ):
            # ---- identity for weight PE transposes ----
            identb = consts.tile([128, 128], BF16)
            make_identity(nc, identb[:])

            # ---- weights: load f32, cast bf16, PE-transpose into combined
            # stationary WhxT[128, j, 98]: cols 0:49 = WhT_j, 49:98 = WxT_j ----
            wnh = consts.tile([R, H], F32, tag="wnh")
            nc.sync.dma_start(out=wnh[:], in_=Wh_d)
            wnx = consts.tile([R, H], F32, tag="wnx")
            nc.sync.dma_start(out=wnx[:], in_=Wx_d)
            wbh = consts.tile([R, H], BF16, tag="wbh")
            nc.vector.tensor_copy(wbh[:], wnh[:])
            wbx = consts.tile([R, H], BF16, tag="wbx")
            nc.vector.tensor_copy(wbx[:], wnx[:])
            tp = ptp.tile([128, 800], BF16, tag="tp")
            for j in range(HT):
                nc.tensor.transpose(
                    tp[:, j * 50 : j * 50 + R],
                    wbh[:, j * 128 : (j + 1) * 128],
                    identb[0:R, 0:R],
                )
                nc.tensor.transpose(
                    tp[:, (HT + j) * 50 : (HT + j) * 50 + R],
                    wbx[:, j * 128 : (j + 1) * 128],
                    identb[0:R, 0:R],
                )
            WhhT = consts.tile([128, HT, PT], BF16, tag="WhhT")
            nc.vector.memset(WhhT[:], 0.0)
            WxxT = consts.tile([128, HT, PT], BF16, tag="WxxT")
            nc.vector.memset(WxxT[:], 0.0)
            tp_ap = tp[:]
            for wt, slot0 in ((WhhT, 0), (WxxT, HT * 50)):
                wt_ap = wt[:]
                nc.vector.tensor_copy(
                    _ap(wt_ap, 0, [wt_ap.ap[0], [PT, HT], [1, R]]),
                    _ap(tp_ap, slot0, [tp_ap.ap[0], [50, HT], [1, R]]),
                )
                nc.vector.tensor_copy(
                    _ap(wt_ap, PB, [wt_ap.ap[0], [PT, HT], [1, R]]),
                    _ap(tp_ap, slot0, [tp_ap.ap[0], [50, HT], [1, R]]),
                )

            prm = consts.tile([128, 16], F32, tag="prm")
            nc.scalar.dma_start(out=prm[:], in_=prm_d)
            prmsum = consts.tile([128, 16], F32, tag="prmsum")
            nc.vector.tensor_copy(prmsum[:], prm[:])

            # ---- Wa as a column vector [49, 1] f32 ----
            WaT = consts.tile([R, 1], F32)
            nc.sync.dma_start(out=WaT[:], in_=_ap(Wa_d, 0, [[1, R], [1, 1]]))

            # ---- per-partition scale/bias const vectors for the 3 groups:
            # rows 0:49 -> params[2g], rows 49:98 -> params[2g+1] ----
            def param_vec(tag, vals):
                vecs = []
                for g in range(NG):
                    v = consts.tile([128, 1], F32, tag=f"{tag}{g}")
                    nc.vector.memset(v[32:PB, :], 0.0)
                    nc.vector.memset(v[96:128, :], 0.0)
                    nc.vector.memset(v[0:R, :], float(vals[2 * g]))
                    nc.vector.memset(v[PB:PT, :], float(vals[2 * g + 1]))
                    vecs.append(v)
                return vecs

            gA = param_vec("gA", FG)
            dA = param_vec("dA", FD)
            gB = param_vec("gB", FGP)
            dB = param_vec("dB", FDP)
            # cwa3[p, g]: rows 0:49 = FC[2g]*Wa, rows 49:98 = FC[2g+1]*Wa
            cwa3 = consts.tile([128, NG], F32, tag="cwa3")
            nc.vector.memset(cwa3[32:PB, :], 0.0)
            nc.vector.memset(cwa3[96:128, :], 0.0)
            for g in range(NG):
                nc.vector.tensor_scalar_mul(
                    cwa3[0:R, g : g + 1], WaT[:], float(FC[2 * g])
                )
                nc.vector.tensor_scalar_mul(
                    cwa3[PB:PT, g : g + 1], WaT[:], float(FC[2 * g + 1])
                )
            # ones column for the softmax denominator matmul (both blocks:
            # bb=0 reads rows 0:49, bb=1 reads rows 64:113 to match xb's
            # base partition)
            onesb = consts.tile([128, 1], BF16, tag="onesb")
            nc.vector.memset(onesb[0:R, :], 1.0)
            nc.vector.memset(onesb[PB : PB + R, :], 1.0)

            # ---- per-pair state (tile versions) ----
            st = [dict() for _ in range(NP)]

            def S_load(p):
                b0 = 2 * p
                xb = xbp.tile([128, H], BF16, tag="xb")
                nc.scalar.dma_start(
                    out=xb[:], in_=_ap(X_d, b0 * KP * H, [[H, 2 * KP], [1, H]])
                )
                hxT = hxT_pool.tile([128, HT, 354], BF16, tag="hxT")
                nc.sync.dma_start(
                    out=hxT[:],
                    in_=_ap(
                        hx_d, p * 128 * HT * 354, [[HT * 354, 128], [1, HT * 354]]
                    ),
                )
                st[p]["xb"] = xb
                st[p]["hxT"] = hxT

            def S_mm1(p):
                cc = pcc.tile([PT, 354], F32, tag="cc")
                hxT = st[p]["hxT"]
                for j in range(HT):
                    nc.tensor.matmul(
                        cc[:, 0:256],
                        WhhT[:, j, :],
                        hxT[:, j, 0:256],
                        start=(j == 0),
                        stop=(j == HT - 1),
                    )
                for j in range(HT):
                    nc.tensor.matmul(
                        cc[:, 256:354],
                        WxxT[:, j, :],
                        hxT[:, j, 256:354],
                        start=(j == 0),
                        stop=(j == HT - 1),
                    )
                st[p]["cc"] = cc

            def S_repl_big(p):
                # single full-width PSUM->SBUF copy (doubled rows already)
                ccr = ccrp.tile([128, 354], F32, tag="ccr")
                cc = st[p]["cc"]
                nc.gpsimd.memset(ccr[32:PB, :], 0.0)
                nc.vector.tensor_copy(ccr[0:PB, :], cc[0:PB, :])
                nc.vector.tensor_copy(ccr[PB:PT, :], cc[PB:PT, :])
                st[p]["ccr"] = ccr

            def S_repl_small(p):
                pass

            def S_tanh(p):
                ccr = st[p]["ccr"]
                SA = SAp.tile([128, NG, 256], BF16, tag="SA")
                for g in range(NG):
                    nc.scalar.activation(
                        SA[0:PT, g, :],
                        ccr[0:PT, 0:256],
                        mybir.ActivationFunctionType.Tanh,
                        bias=dA[g][0:PT, :],
                        scale=gA[g][0:PT, :],
                    )
                sbt = sbtp.tile([128, NG, 2 * R], BF16, tag="sbt")
                for g in range(NG):
                    nc.scalar.activation(
                        sbt[0:PT, g, :],
                        ccr[0:PT, 256:354],
                        mybir.ActivationFunctionType.Tanh,
                        bias=dB[g][0:PT, :],
                        scale=gB[g][0:PT, :],
                    )
                st[p]["SA"] = SA
                st[p]["sbt"] = sbt

            def S_fold(p):
                sbf = sbfp.tile([128, NG, 2 * R], BF16, tag="sbf")
                c3 = cwa3[:]
                nc.gpsimd.tensor_tensor(
                    sbf[0:PT, :, :],
                    st[p]["sbt"][0:PT, :, :],
                    _ap(c3, 0, [[c3.ap[0][0], PT], [1, NG], [0, 2 * R]]),
                    mybir.AluOpType.mult,
                )
                st[p]["sbf"] = sbf

            def S_zT(p):
                zps = psZ.tile([128, 2, 129], F32, tag="z")
                SA = st[p]["SA"]
                sbf = st[p]["sbf"]
                for bb in range(2):
                    for g in range(NG):
                        nc.tensor.matmul(
                            zps[0:R, bb, 0:128],
                            sbf[0:PT, g, bb * R : (bb + 1) * R],
                            SA[0:PT, g, bb * 128 : (bb + 1) * 128],
                            start=(g == 0),
                            stop=(g == NG - 1),
                        )
                st[p]["zps"] = zps

            def S_exp(p):
                ez = ezp.tile([128, 2, 128], BF16, tag="ez")
                nc.scalar.activation(
                    ez[0:R, :, :],
                    st[p]["zps"][0:R, :, 0:128],
                    mybir.ActivationFunctionType.Exp,
                )
                # replicate for bb=1's matmuls (xb block at base partition 64)
                nc.gpsimd.tensor_copy(ez[PB : PB + R, 1, :], ez[0:R, 1, :])
                st[p]["ez"] = ez

            def S_den_mm3(p):
                zps = st[p]["zps"]
                ez = st[p]["ez"]
                xb = st[p]["xb"]
                for bb in range(2):
                    pb = KP * bb
                    nc.tensor.matmul(
                        zps[:, bb, 128:129],
                        ez[pb : pb + R, bb, :],
                        onesb[pb : pb + R, :],
                        start=True,
                        stop=True,
                    )
                rden = rdnp.tile([128, 2], F32, tag="rden")
                nc.vector.reciprocal(rden[:], zps[:, :, 128:129])
                rd = rden[:]
                osb = osbp.tile([128, 2, H], BF16, tag="osb")
                for bb in range(2):
                    pb = KP * bb
                    for hh in range(2):
                        ob = psO.tile([128, 512], F32, tag="ob")
                        nc.tensor.matmul(
                            ob[:],
                            ez[pb : pb + R, bb, :],
                            xb[pb : pb + R, hh * 512 : (hh + 1) * 512],
                            start=True,
                            stop=True,
                        )
                        nc.vector.tensor_tensor(
                            osb[:, bb, hh * 512 : (hh + 1) * 512],
                            ob[:],
                            _ap(rd, bb, [rd.ap[0], [0, 512]]),
                            mybir.AluOpType.mult,
                        )
                st[p]["osb"] = osb

            def S_wr(p):
                nc.scalar.dma_start(
                    out=_ap(out_d, 2 * p * T * H, [[H, T], [T * H, 2], [1, H]]),
                    in_=st[p]["osb"][:],
                )

            # ---- 6-deep software pipeline ----
            for i in range(NP + 5):
                if i >= 5:
                    S_wr(i - 5)
                if 3 <= i < NP + 3:
                    S_zT(i - 3)
                if 4 <= i < NP + 4:
                    S_exp(i - 4)
                    S_den_mm3(i - 4)
                if i < NP:
                    S_load(i)
                if 1 <= i < NP + 1:
                    S_mm1(i - 1)
                if 2 <= i < NP + 2:
                    S_tanh(i - 2)
                    S_fold(i - 2)
                if 1 <= i < NP + 1:
                    S_repl_big(i - 1)
                    S_repl_small(i - 1)

    nc.compile()
    return nc


def _get_nc():
    if "nc" not in _CACHE:
        _CACHE["nc"] = build()
    return _CACHE["nc"]


def make_in_maps(X, h_t, Wx, Wh, Wa):
    import ml_dtypes

    Xf = np.asarray(X, dtype=np.float32)
    Xc = Xf.astype(ml_dtypes.bfloat16)
    Xb = np.zeros((B, KP, H), dtype=ml_dtypes.bfloat16)
    Xb[:, :K, :] = Xc
    hc = np.asarray(h_t, dtype=np.float32).astype(ml_dtypes.bfloat16)
    # pack hxT[pair, p, j, :]: cols 0:256 = h^T (u=bb*128+t), 256:354 = X^T
    NPAIR = B // 2
    hx = np.empty((NPAIR, 128, HT, 354), dtype=ml_dtypes.bfloat16)
    # h part: [pair, bb, t, j, p] -> [pair, p, j, bb*128+t]
    hx[:, :, :, 0:256] = (
        hc.reshape(NPAIR, 2, T, HT, 128)
        .transpose(0, 4, 3, 1, 2)
        .reshape(NPAIR, 128, HT, 256)
    )
    hx[:, :, :, 256:354] = (
        Xc.reshape(NPAIR, 2, K, HT, 128)
        .transpose(0, 4, 3, 1, 2)
        .reshape(NPAIR, 128, HT, 98)
    )
    Wx = np.ascontiguousarray(Wx, dtype=np.float32)
    Wh = np.ascontiguousarray(Wh, dtype=np.float32)
    Wa = np.ascontiguousarray(Wa, dtype=np.float32)
    in_maps = [
        {
            "X": Xb[c * BL : (c + 1) * BL],
            "hxT": np.ascontiguousarray(hx[c * NP : (c + 1) * NP]),
            "Wx": Wx,
            "Wh": Wh,
            "Wa": Wa,
            "prm": np.zeros((128, 16), np.float32),
        }
        for c in range(NCORES)
    ]
    return in_maps


def kernel(X, h_t, Wx, Wh, Wa):
    nc = _get_nc()
    in_maps = make_in_maps(X, h_t, Wx, Wh, Wa)
    last_err = None
    for _attempt in range(3):
        try:
            res = run_bass_kernel_spmd(nc, in_maps, core_ids=list(range(NCORES)))
            break
        except Exception as e:  # transient INTERNAL device errors: retry
            last_err = e
    else:
        raise last_err
    return np.concatenate(
        [np.asarray(res.results[c]["out"]).astype(np.float32) for c in range(NCORES)],
        axis=0,
    )


# revision 3
# speedup vs baseline: 1.0686x; 1.0686x over previous
"""Trainium2 Bass kernel for nn_AttentionBlock: 8-core data-parallel over batch.

Reference computation (per batch b):
  cx = X[b] @ Wx^T               [K,R]   (K=49 regions, R=49, H=1024)
  ch = h_t[b] @ Wh^T             [T,R]   (T=128)
  z[t,k] = sum_r Wa[r] * tanh(cx[k,r] + ch[t,r])
  alpha = softmax_k(z)           [T,K]
  out[b] = alpha @ X[b]          [T,H]

Design (measured 66064 ns vs the 148846 ns v3 baseline):
  - host pre-packs the DMA-heavy layouts:
    * hxT[pair, p, j, :]: bf16 h^T (cols 0:256, u=bb*128+t) and X^T (cols
      256:354, q=bb*49+k) tiles already in matmul-rhs layout -> one large-
      granule DMA per pair, no device transposes or casts at all
    * X padded to 64 rows bf16 (out-matmul rhs; bb=1 block at partition
      base 64, which is a legal engine base)
  - inputs fed bf16, output written bf16 and upcast on host: HBM traffic
    ~12.5MB/core vs 20.4MB all-f32
  - ch/cx via DOUBLED stationaries [Wh|0|Wh], [Wx|0|Wx] built on device:
    cc[113, 0:256]=ch at rows 0:49 AND 64:113, cc[113, 256:354]=cx same;
    gap rows 49:64 are zeros (zero stationary cols)
  - rank-6 tanh-product fit tanh(a+b) ~= sum_m c_m T(g_m a+d_m)T(g'_m b+d'_m)
    (m-pairs packed at partition blocks 0:49 / 64:113, 3 ACT ops per side
    with per-partition scale/bias); z as 3 PSUM-accumulated matmuls per batch
  - z computed TRANSPOSED zT[k, (bb,t)]: softmax needs no PE transpose of
    alpha; no max-subtraction (|z| <= ~6 analytically); denominator via a
    1-column ones-matmul; 1/den folded into the PSUM->SBUF output drain as a
    free-axis-broadcast multiply on DVE
  - 6-deep software pipeline; emission order per iteration only references
    work >= 1 iteration old per engine, so no engine queue head-of-line
    blocks on same-iteration work from another engine
  - PE runs at its high P-state once streams are dense (ramps after ~3us of
    continuous execution; sparse streams run at half clock)
  - pitfalls baked in (each cost a debugging session):
    * engine ops need partition bases that are multiples of 32
    * gpsimd (Pool) cannot touch PSUM
    * the gpsimd memset in S_repl looks redundant but removing it makes the
      NEFF fail at runtime (scheduling artifact); same for odd-inner-dim
      ExternalInputs like [128, 15] f32 -- keep inputs 16/128-padded
    * concurrent XBAR dma_start_transpose ops on both HWDGE rings corrupt
      each other (not used in this version, but measured)
  - kernel() retries run_bass_kernel_spmd up to 3x: the axon device path
    intermittently fails with INTERNAL errors unrelated to the kernel
"""

import sys

sys.path.insert(0, "/opt/trn_rl_repo")

import numpy as np

import concourse.bass as bass
import concourse.bacc as bacc
import concourse.tile as tile
from concourse import mybir
from concourse.bass_utils import run_bass_kernel_spmd
from concourse.masks import make_identity

B, T, K, H = 128, 128, 49, 1024
R = 49
NCORES = 8
BL = B // NCORES  # batches per core
NP = BL // 2  # pairs per core
HT = H // 128  # h tiles
KP = 64  # padded K rows for X (host-side zero pad)
PB = 64  # partition base of the second (m, r) block (must be mult of 32)
PT = PB + R  # 113 partitions used
F32 = mybir.dt.float32
BF16 = mybir.dt.bfloat16

# rank-6 tanh-product fit (LAM=0.03 gauss-weighted, sigma=0.64, A=3.2):
# tanh(a+b) ~= sum_m FC[m] * tanh(FG[m]*a + FD[m]) * tanh(FGP[m]*b + FDP[m])
FG = [0.7368, 2.3523, 1.1871, 2.3100, 0.4495, 1.3332]
FD = [0.0554, 0.1456, -0.8720, 0.1260, -0.3369, -2.8210]
FGP = [-1.3332, -0.4495, 2.3100, 1.1871, 2.3523, 0.7368]
FDP = [-2.8210, -0.3369, -0.1260, 0.8720, -0.1456, -0.0554]
FC = [-1.0581, 1.7567, -0.9840, 0.9840, 1.7567, -1.0581]
NG = 3  # number of (m-pair) partition groups

_CACHE = {}


def _ap(base, off, dims):
    """Custom access pattern on the tensor underlying `base` (an AP)."""
    return bass.AP(tensor=base.tensor, offset=base.offset + off, ap=dims)


def build():
    nc = bacc.Bacc("TRN2", target_bir_lowering=False, debug=False, num_devices=NCORES)

    X_d = nc.dram_tensor("X", [BL, KP, H], BF16, kind="ExternalInput").ap()
    # host-packed transposed rhs tiles: hxT[pair, p, j, 0:256] = h^T block
    # (u = bb*128+t), cols 256:354 = X^T block (q = bb*49+k)
    hx_d = nc.dram_tensor("hxT", [NP, 128, HT, 354], BF16, kind="ExternalInput").ap()
    Wx_d = nc.dram_tensor("Wx", [R, H], F32, kind="ExternalInput").ap()
    Wh_d = nc.dram_tensor("Wh", [R, H], F32, kind="ExternalInput").ap()
    Wa_d = nc.dram_tensor("Wa", [1, R], F32, kind="ExternalInput").ap()
    prm_d = nc.dram_tensor("prm", [128, 16], F32, kind="ExternalInput").ap()
    out_d = nc.dram_tensor("out", [BL, T, H], BF16, kind="ExternalOutput").ap()

    with tile.TileContext(nc) as tc:
        with (
            tc.tile_pool(name="consts", bufs=1) as consts,
            tc.tile_pool(name="hxTp", bufs=3) as hxT_pool,
            tc.tile_pool(name="xbp", bufs=5) as xbp,
            tc.tile_pool(name="ccrp", bufs=3) as ccrp,
            tc.tile_pool(name="SAp", bufs=3) as SAp,
            tc.tile_pool(name="sbtp", bufs=2) as sbtp,
            tc.tile_pool(name="sbfp", bufs=3) as sbfp,
            tc.tile_pool(name="ezp", bufs=3) as ezp,
            tc.tile_pool(name="rdnp", bufs=3) as rdnp,
            tc.tile_pool(name="osbp", bufs=3) as osbp,
            tc.tile_pool(name="ptp", bufs=1, space="PSUM") as ptp,
            tc.tile_pool(name="pcc", bufs=2, space="PSUM") as pcc,
            tc.tile_pool(name="psZ", bufs=2, space="PSUM") as psZ,
            tc.tile_pool(name="psO", bufs=3, space="PSUM") as psO,
        ):
            # ---- identity for weight PE transposes ----
            identb = consts.tile([128, 128], BF16)
            make_identity(nc, identb[:])

            # ---- weights: load f32, cast bf16, PE-transpose into combined
            # stationary WhxT[128, j, 98]: cols 0:49 = WhT_j, 49:98 = WxT_j ----
            wnh = consts.tile([R, H], F32, tag="wnh")
            nc.sync.dma_start(out=wnh[:], in_=Wh_d)
            wnx = consts.tile([R, H], F32, tag="wnx")
            nc.sync.dma_start(out=wnx[:], in_=Wx_d)
            wbh = consts.tile([R, H], BF16, tag="wbh")
            nc.vector.tensor_copy(wbh[:], wnh[:])
            wbx = consts.tile([R, H], BF16, tag="wbx")
            nc.vector.tensor_copy(wbx[:], wnx[:])
            tp = ptp.tile([128, 800], BF16, tag="tp")
            for j in range(HT):
                nc.tensor.transpose(
                    tp[:, j * 50 : j * 50 + R],
                    wbh[:, j * 128 : (j + 1) * 128],
                    identb[0:R, 0:R],
                )
                nc.tensor.transpose(
                    tp[:, (HT + j) * 50 : (HT + j) * 50 + R],
                    wbx[:, j * 128 : (j + 1) * 128],
                    identb[0:R, 0:R],
                )
            WhhT = consts.tile([128, HT, PT], BF16, tag="WhhT")
            nc.vector.memset(WhhT[:], 0.0)
            WxxT = consts.tile([128, HT, PT], BF16, tag="WxxT")
            nc.vector.memset(WxxT[:], 0.0)
            tp_ap = tp[:]
            for wt, slot0 in ((WhhT, 0), (WxxT, HT * 50)):
                wt_ap = wt[:]
                nc.vector.tensor_copy(
                    _ap(wt_ap, 0, [wt_ap.ap[0], [PT, HT], [1, R]]),
                    _ap(tp_ap, slot0, [tp_ap.ap[0], [50, HT], [1, R]]),
                )
                nc.vector.tensor_copy(
                    _ap(wt_ap, PB, [wt_ap.ap[0], [PT, HT], [1, R]]),
                    _ap(tp_ap, slot0, [tp_ap.ap[0], [50, HT], [1, R]]),
                )

            # ones column for the softmax denominator matmul (both blocks:
            # bb=0 reads rows 0:49, bb=1 reads rows 64:113 to match xb's
            # base partition)
            onesb = consts.tile([128, 1], BF16, tag="onesb")
            nc.vector.memset(onesb[0:R, :], 1.0)
            nc.vector.memset(onesb[PB : PB + R, :], 1.0)

            # ---- Wa as a column vector [49, 1] f32 ----
            WaT = consts.tile([R, 1], F32)
            nc.sync.dma_start(out=WaT[:], in_=_ap(Wa_d, 0, [[1, R], [1, 1]]))

            prm = consts.tile([128, 16], F32, tag="prm")
            nc.scalar.dma_start(out=prm[:], in_=prm_d)
            prmsum = consts.tile([128, 16], F32, tag="prmsum")
            nc.vector.tensor_copy(prmsum[:], prm[:])

            # ---- per-partition scale/bias const vectors for the 3 groups:
            # rows 0:49 -> params[2g], rows 49:98 -> params[2g+1] ----
            def param_vec(tag, vals):
                vecs = []
                for g in range(NG):
                    v = consts.tile([128, 1], F32, tag=f"{tag}{g}")
                    nc.vector.memset(v[32:PB, :], 0.0)
                    nc.vector.memset(v[96:128, :], 0.0)
                    nc.vector.memset(v[0:R, :], float(vals[2 * g]))
                    nc.vector.memset(v[PB:PT, :], float(vals[2 * g + 1]))
                    vecs.append(v)
                return vecs

            gA = param_vec("gA", FG)
            dA = param_vec("dA", FD)
            gB = param_vec("gB", FGP)
            dB = param_vec("dB", FDP)
            # cwa3[p, g]: rows 0:49 = FC[2g]*Wa, rows 49:98 = FC[2g+1]*Wa
            cwa3 = consts.tile([128, NG], F32, tag="cwa3")
            nc.vector.memset(cwa3[32:PB, :], 0.0)
            nc.vector.memset(cwa3[96:128, :], 0.0)
            for g in range(NG):
                nc.vector.tensor_scalar_mul(
                    cwa3[0:R, g : g + 1], WaT[:], float(FC[2 * g])
                )
                nc.vector.tensor_scalar_mul(
                    cwa3[PB:PT, g : g + 1], WaT[:], float(FC[2 * g + 1])
                )
            # ---- per-pair state (tile versions) ----
            st = [dict() for _ in range(NP)]

            def S_load(p):
                b0 = 2 * p
                xb = xbp.tile([128, H], BF16, tag="xb")
                nc.scalar.dma_start(
                    out=xb[:], in_=_ap(X_d, b0 * KP * H, [[H, 2 * KP], [1, H]])
                )
                hxT = hxT_pool.tile([128, HT, 354], BF16, tag="hxT")
                nc.sync.dma_start(
                    out=hxT[:],
                    in_=_ap(
                        hx_d, p * 128 * HT * 354, [[HT * 354, 128], [1, HT * 354]]
                    ),
                )
                st[p]["xb"] = xb
                st[p]["hxT"] = hxT

            def S_mm1(p):
                cc = pcc.tile([PT, 354], F32, tag="cc")
                hxT = st[p]["hxT"]
                for j in range(HT):
                    nc.tensor.matmul(
                        cc[:, 0:256],
                        WhhT[:, j, :],
                        hxT[:, j, 0:256],
                        start=(j == 0),
                        stop=(j == HT - 1),
                    )
                for j in range(HT):
                    nc.tensor.matmul(
                        cc[:, 256:354],
                        WxxT[:, j, :],
                        hxT[:, j, 256:354],
                        start=(j == 0),
                        stop=(j == HT - 1),
                    )
                st[p]["cc"] = cc

            def S_repl_big(p):
                # single full-width PSUM->SBUF copy (doubled rows already)
                ccr = ccrp.tile([128, 354], F32, tag="ccr")
                cc = st[p]["cc"]
                nc.gpsimd.memset(ccr[32:PB, :], 0.0)
                nc.vector.tensor_copy(ccr[0:PB, :], cc[0:PB, :])
                nc.vector.tensor_copy(ccr[PB:PT, :], cc[PB:PT, :])
                st[p]["ccr"] = ccr

            def S_repl_small(p):
                pass

            def S_tanh(p):
                ccr = st[p]["ccr"]
                SA = SAp.tile([128, NG, 256], BF16, tag="SA")
                for g in range(NG):
                    nc.scalar.activation(
                        SA[0:PT, g, :],
                        ccr[0:PT, 0:256],
                        mybir.ActivationFunctionType.Tanh,
                        bias=dA[g][0:PT, :],
                        scale=gA[g][0:PT, :],
                    )
                sbt = sbtp.tile([128, NG, 2 * R], BF16, tag="sbt")
                for g in range(NG):
                    nc.scalar.activation(
                        sbt[0:PT, g, :],
                        ccr[0:PT, 256:354],
                        mybir.ActivationFunctionType.Tanh,
                        bias=dB[g][0:PT, :],
                        scale=gB[g][0:PT, :],
                    )
                st[p]["SA"] = SA
                st[p]["sbt"] = sbt

            def S_fold(p):
                sbf = sbfp.tile([128, NG, 2 * R], BF16, tag="sbf")
                c3 = cwa3[:]
                nc.gpsimd.tensor_tensor(
                    sbf[0:PT, :, :],
                    st[p]["sbt"][0:PT, :, :],
                    _ap(c3, 0, [[c3.ap[0][0], PT], [1, NG], [0, 2 * R]]),
                    mybir.AluOpType.mult,
                )
                st[p]["sbf"] = sbf

            def S_zT(p):
                zps = psZ.tile([128, 2, 129], F32, tag="z")
                SA = st[p]["SA"]
                sbf = st[p]["sbf"]
                for bb in range(2):
                    for g in range(NG):
                        nc.tensor.matmul(
                            zps[0:R, bb, 0:128],
                            sbf[0:PT, g, bb * R : (bb + 1) * R],
                            SA[0:PT, g, bb * 128 : (bb + 1) * 128],
                            start=(g == 0),
                            stop=(g == NG - 1),
                        )
                st[p]["zps"] = zps

            def S_exp(p):
                ez = ezp.tile([128, 2, 128], BF16, tag="ez")
                nc.scalar.activation(
                    ez[0:R, :, :],
                    st[p]["zps"][0:R, :, 0:128],
                    mybir.ActivationFunctionType.Exp,
                )
                # replicate for bb=1's matmuls (xb block at base partition 64)
                nc.gpsimd.tensor_copy(ez[PB : PB + R, 1, :], ez[0:R, 1, :])
                st[p]["ez"] = ez

            def S_den_mm3(p):
                zps = st[p]["zps"]
                ez = st[p]["ez"]
                xb = st[p]["xb"]
                for bb in range(2):
                    pb = KP * bb
                    nc.tensor.matmul(
                        zps[:, bb, 128:129],
                        ez[pb : pb + R, bb, :],
                        onesb[pb : pb + R, :],
                        start=True,
                        stop=True,
                    )
                rden = rdnp.tile([128, 2], F32, tag="rden")
                nc.vector.reciprocal(rden[:], zps[:, :, 128:129])
                rd = rden[:]
                osb = osbp.tile([128, 2, H], BF16, tag="osb")
                for bb in range(2):
                    pb = KP * bb
                    for hh in range(2):
                        ob = psO.tile([128, 512], F32, tag="ob")
                        nc.tensor.matmul(
                            ob[:],
                            ez[pb : pb + R, bb, :],
                            xb[pb : pb + R, hh * 512 : (hh + 1) * 512],
                            start=True,
                            stop=True,
                        )
                        nc.vector.tensor_tensor(
                            osb[:, bb, hh * 512 : (hh + 1) * 512],
                            ob[:],
                            _ap(rd, bb, [rd.ap[0], [0, 512]]),
                            mybir.AluOpType.mult,
                        )
                st[p]["osb"] = osb

            def S_wr(p):
                nc.scalar.dma_start(
                    out=_ap(out_d, 2 * p * T * H, [[H, T], [T * H, 2], [1, H]]),
                    in_=st[p]["osb"][:],
                )

            # ---- 6-deep software pipeline ----
            for i in range(NP + 5):
                if i >= 5:
                    S_wr(i - 5)
                if 4 <= i < NP + 4:
                    S_exp(i - 4)
                    S_den_mm3(i - 4)
                if 3 <= i < NP + 3:
                    S_zT(i - 3)
                if i < NP:
                    S_load(i)
                if 1 <= i < NP + 1:
                    S_mm1(i - 1)
                if 2 <= i < NP + 2:
                    S_tanh(i - 2)
                    S_fold(i - 2)
                if 1 <= i < NP + 1:
                    S_repl_big(i - 1)
                    S_repl_small(i - 1)

    nc.compile()
    return nc


def _get_nc():
    if "nc" not in _CACHE:
        _CACHE["nc"] = build()
    return _CACHE["nc"]


def make_in_maps(X, h_t, Wx, Wh, Wa):
    import ml_dtypes

    Xf = np.asarray(X, dtype=np.float32)
    Xc = Xf.astype(ml_dtypes.bfloat16)
    Xb = np.zeros((B, KP, H), dtype=ml_dtypes.bfloat16)
    Xb[:, :K, :] = Xc
    hc = np.asarray(h_t, dtype=np.float32).astype(ml_dtypes.bfloat16)
    # pack hxT[pair, p, j, :]: cols 0:256 = h^T (u=bb*128+t), 256:354 = X^T
    NPAIR = B // 2
    hx = np.empty((NPAIR, 128, HT, 354), dtype=ml_dtypes.bfloat16)
    # h part: [pair, bb, t, j, p] -> [pair, p, j, bb*128+t]
    hx[:, :, :, 0:256] = (
        hc.reshape(NPAIR, 2, T, HT, 128)
        .transpose(0, 4, 3, 1, 2)
        .reshape(NPAIR, 128, HT, 256)
    )
    hx[:, :, :, 256:354] = (
        Xc.reshape(NPAIR, 2, K, HT, 128)
        .transpose(0, 4, 3, 1, 2)
        .reshape(NPAIR, 128, HT, 98)
    )
    Wx = np.ascontiguousarray(Wx, dtype=np.float32)
    Wh = np.ascontiguousarray(Wh, dtype=np.float32)
    Wa = np.ascontiguousarray(Wa, dtype=np.float32)
    in_maps = [
        {
            "X": Xb[c * BL : (c + 1) * BL],
            "hxT": np.ascontiguousarray(hx[c * NP : (c + 1) * NP]),
            "Wx": Wx,
            "Wh": Wh,
            "Wa": Wa,
            "prm": np.zeros((128, 16), np.float32),
        }
        for c in range(NCORES)
    ]
    return in_maps


def kernel(X, h_t, Wx, Wh, Wa):
    nc = _get_nc()
    in_maps = make_in_maps(X, h_t, Wx, Wh, Wa)
    last_err = None
    for _attempt in range(3):
        try:
            res = run_bass_kernel_spmd(nc, in_maps, core_ids=list(range(NCORES)))
            break
        except Exception as e:  # transient INTERNAL device errors: retry
            last_err = e
    else:
        raise last_err
    return np.concatenate(
        [np.asarray(res.results[c]["out"]).astype(np.float32) for c in range(NCORES)],
        axis=0,
    )


# revision 4
# speedup vs baseline: 1.0950x; 1.0247x over previous
"""Trainium2 Bass kernel for nn_AttentionBlock: 8-core data-parallel over batch.

Reference computation (per batch b):
  cx = X[b] @ Wx^T               [K,R]   (K=49 regions, R=49, H=1024)
  ch = h_t[b] @ Wh^T             [T,R]   (T=128)
  z[t,k] = sum_r Wa[r] * tanh(cx[k,r] + ch[t,r])
  alpha = softmax_k(z)           [T,K]
  out[b] = alpha @ X[b]          [T,H]

Design (measured ~66us median vs the 148846 ns v3 baseline):
  - host pre-packs the DMA-heavy layouts: hxT[pair, p, j, :] holds bf16 h^T
    (cols 0:256, u=bb*128+t) and X^T (cols 256:354, q=bb*49+k) already in
    matmul-rhs layout -> one large-granule DMA per pair, zero device-side
    transposes or casts; X padded to 64 rows bf16 (partition-base-64 legal)
  - inputs fed bf16, output written bf16 and upcast on host: HBM traffic
    ~12.5MB/core vs 20.4MB all-f32
  - ch/cx via DOUBLED stationaries [Wh|0|Wh], [Wx|0|Wx] built on device:
    cc[113, 0:256]=ch at rows 0:49 AND 64:113, cc[113, 256:354]=cx same;
    gap rows 49:64 are zeros (zero stationary cols)
  - rank-6 tanh-product fit tanh(a+b) ~= sum_m c_m T(g_m a+d_m)T(g'_m b+d'_m),
    m-pairs packed at partition blocks 0:49 / 64:113, 3 ACT ops per side with
    per-partition scale/bias; z as 3 PSUM-accumulated matmuls per batch
  - z computed TRANSPOSED zT[k, (bb,t)]: no alpha transpose, no
    max-subtraction (|z| <= ~6 analytically); denominator via a 1-column
    ones-matmul; 1/den folded into the PSUM->SBUF output drain on DVE
  - 6-deep software pipeline; per-iteration emission only references work
    >= 1 iteration old per engine (no head-of-line blocking); PE ramps to
    its 2.4GHz P-state once streams are dense (~3us continuous)
  - pitfalls baked in (each cost a debugging session):
    * engine ops need partition bases that are multiples of 32
    * gpsimd (Pool) cannot touch PSUM; bf16 memset into PSUM is illegal
    * the gpsimd memset in S_repl looks redundant but removing it makes the
      NEFF fail at runtime (scheduling artifact); odd-inner-dim inputs like
      [128, 15] f32 also crash -- keep ExternalInputs 16/128-padded
    * concurrent XBAR dma_start_transpose on both HWDGE rings corrupt each
      other (not used here, but measured)
  - kernel() retries run_bass_kernel_spmd up to 3x: the axon device path
    intermittently fails with INTERNAL errors unrelated to the kernel
"""

import sys

sys.path.insert(0, "/opt/trn_rl_repo")

import numpy as np

import concourse.bass as bass
import concourse.bacc as bacc
import concourse.tile as tile
from concourse import mybir
from concourse.bass_utils import run_bass_kernel_spmd
from concourse.masks import make_identity

B, T, K, H = 128, 128, 49, 1024
R = 49
NCORES = 8
BL = B // NCORES  # batches per core
NP = BL // 2  # pairs per core
HT = H // 128  # h tiles
KP = 64  # padded K rows for X (host-side zero pad)
PB = 64  # partition base of the second (m, r) block (must be mult of 32)
PT = PB + R  # 113 partitions used
F32 = mybir.dt.float32
BF16 = mybir.dt.bfloat16

# rank-6 tanh-product fit (LAM=0.03 gauss-weighted, sigma=0.64, A=3.2):
# tanh(a+b) ~= sum_m FC[m] * tanh(FG[m]*a + FD[m]) * tanh(FGP[m]*b + FDP[m])
FG = [0.7368, 2.3523, 1.1871, 2.3100, 0.4495, 1.3332]
FD = [0.0554, 0.1456, -0.8720, 0.1260, -0.3369, -2.8210]
FGP = [-1.3332, -0.4495, 2.3100, 1.1871, 2.3523, 0.7368]
FDP = [-2.8210, -0.3369, -0.1260, 0.8720, -0.1456, -0.0554]
FC = [-1.0581, 1.7567, -0.9840, 0.9840, 1.7567, -1.0581]
NG = 3  # number of (m-pair) partition groups

_CACHE = {}


def _ap(base, off, dims):
    """Custom access pattern on the tensor underlying `base` (an AP)."""
    return bass.AP(tensor=base.tensor, offset=base.offset + off, ap=dims)


def build():
    nc = bacc.Bacc("TRN2", target_bir_lowering=False, debug=False, num_devices=NCORES)

    X_d = nc.dram_tensor("X", [BL, KP, H], BF16, kind="ExternalInput").ap()
    # host-packed transposed rhs tiles: hxT[pair, p, j, 0:256] = h^T block
    # (u = bb*128+t), cols 256:354 = X^T block (q = bb*49+k)
    hx_d = nc.dram_tensor("hxT", [NP, 128, HT, 354], BF16, kind="ExternalInput").ap()
    Wx_d = nc.dram_tensor("Wx", [R, H], F32, kind="ExternalInput").ap()
    Wh_d = nc.dram_tensor("Wh", [R, H], F32, kind="ExternalInput").ap()
    Wa_d = nc.dram_tensor("Wa", [1, R], F32, kind="ExternalInput").ap()
    prm_d = nc.dram_tensor("prm", [128, 16], F32, kind="ExternalInput").ap()
    out_d = nc.dram_tensor("out", [BL, T, H], BF16, kind="ExternalOutput").ap()

    with tile.TileContext(nc) as tc:
        with (
            tc.tile_pool(name="consts", bufs=1) as consts,
            tc.tile_pool(name="hxTp", bufs=3) as hxT_pool,
            tc.tile_pool(name="xbp", bufs=5) as xbp,
            tc.tile_pool(name="ccrp", bufs=3) as ccrp,
            tc.tile_pool(name="SAp", bufs=3) as SAp,
            tc.tile_pool(name="sbtp", bufs=2) as sbtp,
            tc.tile_pool(name="sbfp", bufs=3) as sbfp,
            tc.tile_pool(name="ezp", bufs=3) as ezp,
            tc.tile_pool(name="rdnp", bufs=3) as rdnp,
            tc.tile_pool(name="osbp", bufs=3) as osbp,
            tc.tile_pool(name="ptp", bufs=1, space="PSUM") as ptp,
            tc.tile_pool(name="pcc", bufs=2, space="PSUM") as pcc,
            tc.tile_pool(name="psZ", bufs=2, space="PSUM") as psZ,
            tc.tile_pool(name="psO", bufs=3, space="PSUM") as psO,
        ):
            # ---- identity for weight PE transposes ----
            identb = consts.tile([128, 128], BF16)
            make_identity(nc, identb[:])

            # ---- weights: load f32, cast bf16, PE-transpose into combined
            # stationary WhxT[128, j, 98]: cols 0:49 = WhT_j, 49:98 = WxT_j ----
            wnh = consts.tile([R, H], F32, tag="wnh")
            nc.sync.dma_start(out=wnh[:], in_=Wh_d)
            wnx = consts.tile([R, H], F32, tag="wnx")
            nc.sync.dma_start(out=wnx[:], in_=Wx_d)
            wbh = consts.tile([R, H], BF16, tag="wbh")
            nc.vector.tensor_copy(wbh[:], wnh[:])
            wbx = consts.tile([R, H], BF16, tag="wbx")
            nc.vector.tensor_copy(wbx[:], wnx[:])
            tp = ptp.tile([128, 800], BF16, tag="tp")
            for j in range(HT):
                nc.tensor.transpose(
                    tp[:, j * 50 : j * 50 + R],
                    wbh[:, j * 128 : (j + 1) * 128],
                    identb[0:R, 0:R],
                )
                nc.tensor.transpose(
                    tp[:, (HT + j) * 50 : (HT + j) * 50 + R],
                    wbx[:, j * 128 : (j + 1) * 128],
                    identb[0:R, 0:R],
                )
            WhhT = consts.tile([128, HT, PT], BF16, tag="WhhT")
            nc.vector.memset(WhhT[:], 0.0)
            WxxT = consts.tile([128, HT, PT], BF16, tag="WxxT")
            nc.vector.memset(WxxT[:], 0.0)
            tp_ap = tp[:]
            for wt, slot0 in ((WhhT, 0), (WxxT, HT * 50)):
                wt_ap = wt[:]
                nc.vector.tensor_copy(
                    _ap(wt_ap, 0, [wt_ap.ap[0], [PT, HT], [1, R]]),
                    _ap(tp_ap, slot0, [tp_ap.ap[0], [50, HT], [1, R]]),
                )
                nc.vector.tensor_copy(
                    _ap(wt_ap, PB, [wt_ap.ap[0], [PT, HT], [1, R]]),
                    _ap(tp_ap, slot0, [tp_ap.ap[0], [50, HT], [1, R]]),
                )

            prm = consts.tile([128, 16], F32, tag="prm")
            nc.scalar.dma_start(out=prm[:], in_=prm_d)
            prmsum = consts.tile([128, 16], F32, tag="prmsum")
            nc.vector.tensor_copy(prmsum[:], prm[:])

            # ---- Wa as a column vector [49, 1] f32 ----
            WaT = consts.tile([R, 1], F32)
            nc.sync.dma_start(out=WaT[:], in_=_ap(Wa_d, 0, [[1, R], [1, 1]]))

            # ---- per-partition scale/bias const vectors for the 3 groups:
            # rows 0:49 -> params[2g], rows 49:98 -> params[2g+1] ----
            def param_vec(tag, vals):
                vecs = []
                for g in range(NG):
                    v = consts.tile([128, 1], F32, tag=f"{tag}{g}")
                    nc.vector.memset(v[32:PB, :], 0.0)
                    nc.vector.memset(v[96:128, :], 0.0)
                    nc.vector.memset(v[0:R, :], float(vals[2 * g]))
                    nc.vector.memset(v[PB:PT, :], float(vals[2 * g + 1]))
                    vecs.append(v)
                return vecs

            gA = param_vec("gA", FG)
            dA = param_vec("dA", FD)
            gB = param_vec("gB", FGP)
            dB = param_vec("dB", FDP)
            # cwa3[p, g]: rows 0:49 = FC[2g]*Wa, rows 49:98 = FC[2g+1]*Wa
            cwa3 = consts.tile([128, NG], F32, tag="cwa3")
            nc.vector.memset(cwa3[32:PB, :], 0.0)
            nc.vector.memset(cwa3[96:128, :], 0.0)
            for g in range(NG):
                nc.vector.tensor_scalar_mul(
                    cwa3[0:R, g : g + 1], WaT[:], float(FC[2 * g])
                )
                nc.vector.tensor_scalar_mul(
                    cwa3[PB:PT, g : g + 1], WaT[:], float(FC[2 * g + 1])
                )
            # ones column for the softmax denominator matmul (both blocks:
            # bb=0 reads rows 0:49, bb=1 reads rows 64:113 to match xb's
            # base partition)
            onesb = consts.tile([128, 1], BF16, tag="onesb")
            nc.vector.memset(onesb[0:R, :], 1.0)
            nc.vector.memset(onesb[PB : PB + R, :], 1.0)

            # ---- per-pair state (tile versions) ----
            st = [dict() for _ in range(NP)]

            def S_load(p):
                b0 = 2 * p
                xb = xbp.tile([128, H], BF16, tag="xb")
                nc.scalar.dma_start(
                    out=xb[:], in_=_ap(X_d, b0 * KP * H, [[H, 2 * KP], [1, H]])
                )
                hxT = hxT_pool.tile([128, HT, 354], BF16, tag="hxT")
                nc.sync.dma_start(
                    out=hxT[:],
                    in_=_ap(
                        hx_d, p * 128 * HT * 354, [[HT * 354, 128], [1, HT * 354]]
                    ),
                )
                st[p]["xb"] = xb
                st[p]["hxT"] = hxT

            def S_mm1(p):
                cc = pcc.tile([PT, 354], F32, tag="cc")
                hxT = st[p]["hxT"]
                for j in range(HT):
                    nc.tensor.matmul(
                        cc[:, 0:256],
                        WhhT[:, j, :],
                        hxT[:, j, 0:256],
                        start=(j == 0),
                        stop=(j == HT - 1),
                    )
                for j in range(HT):
                    nc.tensor.matmul(
                        cc[:, 256:354],
                        WxxT[:, j, :],
                        hxT[:, j, 256:354],
                        start=(j == 0),
                        stop=(j == HT - 1),
                    )
                st[p]["cc"] = cc

            def S_repl_big(p):
                # single full-width PSUM->SBUF copy (doubled rows already)
                ccr = ccrp.tile([128, 354], F32, tag="ccr")
                cc = st[p]["cc"]
                nc.gpsimd.memset(ccr[32:PB, :], 0.0)
                nc.vector.tensor_copy(ccr[0:PB, :], cc[0:PB, :])
                nc.vector.tensor_copy(ccr[PB:PT, :], cc[PB:PT, :])
                st[p]["ccr"] = ccr

            def S_repl_small(p):
                pass

            def S_tanh(p):
                ccr = st[p]["ccr"]
                SA = SAp.tile([128, NG, 256], BF16, tag="SA")
                for g in range(NG):
                    nc.scalar.activation(
                        SA[0:PT, g, :],
                        ccr[0:PT, 0:256],
                        mybir.ActivationFunctionType.Tanh,
                        bias=dA[g][0:PT, :],
                        scale=gA[g][0:PT, :],
                    )
                sbt = sbtp.tile([128, NG, 2 * R], BF16, tag="sbt")
                for g in range(NG):
                    nc.scalar.activation(
                        sbt[0:PT, g, :],
                        ccr[0:PT, 256:354],
                        mybir.ActivationFunctionType.Tanh,
                        bias=dB[g][0:PT, :],
                        scale=gB[g][0:PT, :],
                    )
                st[p]["SA"] = SA
                st[p]["sbt"] = sbt

            def S_fold(p):
                sbf = sbfp.tile([128, NG, 2 * R], BF16, tag="sbf")
                c3 = cwa3[:]
                nc.gpsimd.tensor_tensor(
                    sbf[0:PT, :, :],
                    st[p]["sbt"][0:PT, :, :],
                    _ap(c3, 0, [[c3.ap[0][0], PT], [1, NG], [0, 2 * R]]),
                    mybir.AluOpType.mult,
                )
                st[p]["sbf"] = sbf

            def S_zT(p):
                zps = psZ.tile([128, 2, 129], F32, tag="z")
                SA = st[p]["SA"]
                sbf = st[p]["sbf"]
                for bb in range(2):
                    for g in range(NG):
                        nc.tensor.matmul(
                            zps[0:R, bb, 0:128],
                            sbf[0:PT, g, bb * R : (bb + 1) * R],
                            SA[0:PT, g, bb * 128 : (bb + 1) * 128],
                            start=(g == 0),
                            stop=(g == NG - 1),
                        )
                st[p]["zps"] = zps

            def S_exp(p):
                ez = ezp.tile([128, 2, 128], BF16, tag="ez")
                nc.scalar.activation(
                    ez[0:R, :, :],
                    st[p]["zps"][0:R, :, 0:128],
                    mybir.ActivationFunctionType.Exp,
                )
                # replicate for bb=1's matmuls (xb block at base partition 64)
                nc.gpsimd.tensor_copy(ez[PB : PB + R, 1, :], ez[0:R, 1, :])
                st[p]["ez"] = ez

            def S_den_mm3(p):
                zps = st[p]["zps"]
                ez = st[p]["ez"]
                xb = st[p]["xb"]
                for bb in range(2):
                    pb = KP * bb
                    nc.tensor.matmul(
                        zps[:, bb, 128:129],
                        ez[pb : pb + R, bb, :],
                        onesb[pb : pb + R, :],
                        start=True,
                        stop=True,
                    )
                rden = rdnp.tile([128, 2], F32, tag="rden")
                nc.vector.reciprocal(rden[:], zps[:, :, 128:129])
                rd = rden[:]
                osb = osbp.tile([128, 2, H], BF16, tag="osb")
                for bb in range(2):
                    pb = KP * bb
                    for hh in range(2):
                        ob = psO.tile([128, 512], F32, tag="ob")
                        nc.tensor.matmul(
                            ob[:],
                            ez[pb : pb + R, bb, :],
                            xb[pb : pb + R, hh * 512 : (hh + 1) * 512],
                            start=True,
                            stop=True,
                        )
                        nc.vector.tensor_tensor(
                            osb[:, bb, hh * 512 : (hh + 1) * 512],
                            ob[:],
                            _ap(rd, bb, [rd.ap[0], [0, 512]]),
                            mybir.AluOpType.mult,
                        )
                st[p]["osb"] = osb

            def S_wr(p):
                nc.scalar.dma_start(
                    out=_ap(out_d, 2 * p * T * H, [[H, T], [T * H, 2], [1, H]]),
                    in_=st[p]["osb"][:],
                )

            # ---- 6-deep software pipeline ----
            for i in range(NP + 5):
                if i >= 5:
                    S_wr(i - 5)
                if 3 <= i < NP + 3:
                    S_zT(i - 3)
                if 4 <= i < NP + 4:
                    S_exp(i - 4)
                    S_den_mm3(i - 4)
                if i < NP:
                    S_load(i)
                if 1 <= i < NP + 1:
                    S_mm1(i - 1)
                if 2 <= i < NP + 2:
                    S_tanh(i - 2)
                    S_fold(i - 2)
                if 1 <= i < NP + 1:
                    S_repl_big(i - 1)
                    S_repl_small(i - 1)

    nc.compile()
    return nc


def _get_nc():
    if "nc" not in _CACHE:
        _CACHE["nc"] = build()
    return _CACHE["nc"]


def make_in_maps(X, h_t, Wx, Wh, Wa):
    import ml_dtypes

    Xf = np.asarray(X, dtype=np.float32)
    Xc = Xf.astype(ml_dtypes.bfloat16)
    Xb = np.zeros((B, KP, H), dtype=ml_dtypes.bfloat16)
    Xb[:, :K, :] = Xc
    hc = np.asarray(h_t, dtype=np.float32).astype(ml_dtypes.bfloat16)
    # pack hxT[pair, p, j, :]: cols 0:256 = h^T (u=bb*128+t), 256:354 = X^T
    NPAIR = B // 2
    hx = np.empty((NPAIR, 128, HT, 354), dtype=ml_dtypes.bfloat16)
    # h part: [pair, bb, t, j, p] -> [pair, p, j, bb*128+t]
    hx[:, :, :, 0:256] = (
        hc.reshape(NPAIR, 2, T, HT, 128)
        .transpose(0, 4, 3, 1, 2)
        .reshape(NPAIR, 128, HT, 256)
    )
    hx[:, :, :, 256:354] = (
        Xc.reshape(NPAIR, 2, K, HT, 128)
        .transpose(0, 4, 3, 1, 2)
        .reshape(NPAIR, 128, HT, 98)
    )
    Wx = np.ascontiguousarray(Wx, dtype=np.float32)
    Wh = np.ascontiguousarray(Wh, dtype=np.float32)
    Wa = np.ascontiguousarray(Wa, dtype=np.float32)
    in_maps = [
        {
            "X": Xb[c * BL : (c + 1) * BL],
            "hxT": np.ascontiguousarray(hx[c * NP : (c + 1) * NP]),
            "Wx": Wx,
            "Wh": Wh,
            "Wa": Wa,
            "prm": np.zeros((128, 16), np.float32),
        }
        for c in range(NCORES)
    ]
    return in_maps


def kernel(X, h_t, Wx, Wh, Wa):
    nc = _get_nc()
    in_maps = make_in_maps(X, h_t, Wx, Wh, Wa)
    last_err = None
    for _attempt in range(3):
        try:
            res = run_bass_kernel_spmd(nc, in_maps, core_ids=list(range(NCORES)))
            break
        except Exception as e:  # transient INTERNAL device errors: retry
            last_err = e
    else:
        raise last_err
    return np.concatenate(
        [np.asarray(res.results[c]["out"]).astype(np.float32) for c in range(NCORES)],
        axis=0,
    )


# revision 5
# speedup vs baseline: 1.1114x; 1.0150x over previous
"""Trainium2 Bass kernel for nn_AttentionBlock: 8-core data-parallel over batch.

Reference computation (per batch b):
  cx = X[b] @ Wx^T               [K,R]   (K=49 regions, R=49, H=1024)
  ch = h_t[b] @ Wh^T             [T,R]   (T=128)
  z[t,k] = sum_r Wa[r] * tanh(cx[k,r] + ch[t,r])
  alpha = softmax_k(z)           [T,K]
  out[b] = alpha @ X[b]          [T,H]

Design (measured ~66us median vs the 148846 ns v3 baseline):
  - host pre-packs the DMA-heavy layouts: hxT[pair, p, j, :] holds bf16 h^T
    (cols 0:256, u=bb*128+t) and X^T (cols 256:354, q=bb*49+k) already in
    matmul-rhs layout -> one large-granule DMA per pair, zero device-side
    transposes or casts; X padded to 64 rows bf16 (partition-base-64 legal)
  - inputs fed bf16, output written bf16 and upcast on host: HBM traffic
    ~12.5MB/core vs 20.4MB all-f32
  - ch/cx via DOUBLED stationaries [Wh|0|Wh], [Wx|0|Wx] built on device:
    cc[113, 0:256]=ch at rows 0:49 AND 64:113, cc[113, 256:354]=cx same;
    gap rows 49:64 are zeros (zero stationary cols)
  - rank-6 tanh-product fit tanh(a+b) ~= sum_m c_m T(g_m a+d_m)T(g'_m b+d'_m),
    m-pairs packed at partition blocks 0:49 / 64:113, 3 ACT ops per side with
    per-partition scale/bias; z as 3 PSUM-accumulated matmuls per batch
  - z computed TRANSPOSED zT[k, (bb,t)]: no alpha transpose, no
    max-subtraction (|z| <= ~6 analytically); denominator via a 1-column
    ones-matmul; 1/den folded into the PSUM->SBUF output drain on DVE
  - 6-deep software pipeline; per-iteration emission only references work
    >= 1 iteration old per engine (no head-of-line blocking); PE ramps to
    its 2.4GHz P-state once streams are dense (~3us continuous)
  - pitfalls baked in (each cost a debugging session):
    * engine ops need partition bases that are multiples of 32
    * gpsimd (Pool) cannot touch PSUM; bf16 memset into PSUM is illegal
    * the gpsimd memset in S_repl looks redundant but removing it makes the
      NEFF fail at runtime (scheduling artifact); odd-inner-dim inputs like
      [128, 15] f32 also crash -- keep ExternalInputs 16/128-padded
    * concurrent XBAR dma_start_transpose on both HWDGE rings corrupt each
      other (not used here, but measured)
  - kernel() retries run_bass_kernel_spmd up to 3x: the axon device path
    intermittently fails with INTERNAL errors unrelated to the kernel
"""

import sys

sys.path.insert(0, "/opt/trn_rl_repo")

import numpy as np

import concourse.bass as bass
import concourse.bacc as bacc
import concourse.tile as tile
from concourse import mybir
from concourse.bass_utils import run_bass_kernel_spmd
from concourse.masks import make_identity

B, T, K, H = 128, 128, 49, 1024
R = 49
NCORES = 8
BL = B // NCORES  # batches per core
NP = BL // 2  # pairs per core
HT = H // 128  # h tiles
KP = 64  # padded K rows for X (host-side zero pad)
PB = 64  # partition base of the second (m, r) block (must be mult of 32)
PT = PB + R  # 113 partitions used
F32 = mybir.dt.float32
BF16 = mybir.dt.bfloat16

# rank-6 tanh-product fit (LAM=0.03 gauss-weighted, sigma=0.64, A=3.2):
# tanh(a+b) ~= sum_m FC[m] * tanh(FG[m]*a + FD[m]) * tanh(FGP[m]*b + FDP[m])
# rank-4 refit (end-to-end 6.2e-3 incl bf16 vs rank-6's 3.7e-3; gate 2e-2)
FG = [0.9794, -0.9901, 1.0111, -0.0645]
FD = [-0.1872, 0.0508, 3.1349, 0.0117]
FGP = [0.9891, -0.9785, -0.0725, -1.013]
FDP = [0.0506, -0.1871, -0.0132, 3.1413]
FC = [-5.0578, 5.0574, -3.2168, -3.6059]
NG = 2  # number of (m-pair) partition groups

_CACHE = {}


def _ap(base, off, dims):
    """Custom access pattern on the tensor underlying `base` (an AP)."""
    return bass.AP(tensor=base.tensor, offset=base.offset + off, ap=dims)


def build():
    nc = bacc.Bacc("TRN2", target_bir_lowering=False, debug=False, num_devices=NCORES)

    X_d = nc.dram_tensor("X", [BL, KP, H], BF16, kind="ExternalInput").ap()
    # host-packed transposed rhs tiles: hxT[pair, p, j, 0:256] = h^T block
    # (u = bb*128+t), cols 256:354 = X^T block (q = bb*49+k)
    hx_d = nc.dram_tensor("hxT", [NP, 128, HT, 354], BF16, kind="ExternalInput").ap()
    Wx_d = nc.dram_tensor("Wx", [R, H], F32, kind="ExternalInput").ap()
    Wh_d = nc.dram_tensor("Wh", [R, H], F32, kind="ExternalInput").ap()
    Wa_d = nc.dram_tensor("Wa", [1, R], F32, kind="ExternalInput").ap()
    prm_d = nc.dram_tensor("prm", [128, 16], F32, kind="ExternalInput").ap()
    out_d = nc.dram_tensor("out", [BL, T, H], BF16, kind="ExternalOutput").ap()

    with tile.TileContext(nc) as tc:
        with (
            tc.tile_pool(name="consts", bufs=1) as consts,
            tc.tile_pool(name="hxTp", bufs=3) as hxT_pool,
            tc.tile_pool(name="xbp", bufs=5) as xbp,
            tc.tile_pool(name="ccrp", bufs=3) as ccrp,
            tc.tile_pool(name="SAp", bufs=3) as SAp,
            tc.tile_pool(name="sbtp", bufs=2) as sbtp,
            tc.tile_pool(name="sbfp", bufs=3) as sbfp,
            tc.tile_pool(name="ezp", bufs=3) as ezp,
            tc.tile_pool(name="rdnp", bufs=3) as rdnp,
            tc.tile_pool(name="osbp", bufs=3) as osbp,
            tc.tile_pool(name="ptp", bufs=1, space="PSUM") as ptp,
            tc.tile_pool(name="pcc", bufs=2, space="PSUM") as pcc,
            tc.tile_pool(name="psZ", bufs=2, space="PSUM") as psZ,
            tc.tile_pool(name="psO", bufs=3, space="PSUM") as psO,
        ):
            # ---- identity for weight PE transposes ----
            identb = consts.tile([128, 128], BF16)
            make_identity(nc, identb[:])

            # ---- weights: load f32, cast bf16, PE-transpose into combined
            # stationary WhxT[128, j, 98]: cols 0:49 = WhT_j, 49:98 = WxT_j ----
            wnh = consts.tile([R, H], F32, tag="wnh")
            nc.sync.dma_start(out=wnh[:], in_=Wh_d)
            wnx = consts.tile([R, H], F32, tag="wnx")
            nc.sync.dma_start(out=wnx[:], in_=Wx_d)
            wbh = consts.tile([R, H], BF16, tag="wbh")
            nc.vector.tensor_copy(wbh[:], wnh[:])
            wbx = consts.tile([R, H], BF16, tag="wbx")
            nc.vector.tensor_copy(wbx[:], wnx[:])
            tp = ptp.tile([128, 800], BF16, tag="tp")
            for j in range(HT):
                nc.tensor.transpose(
                    tp[:, j * 50 : j * 50 + R],
                    wbh[:, j * 128 : (j + 1) * 128],
                    identb[0:R, 0:R],
                )
                nc.tensor.transpose(
                    tp[:, (HT + j) * 50 : (HT + j) * 50 + R],
                    wbx[:, j * 128 : (j + 1) * 128],
                    identb[0:R, 0:R],
                )
            WhhT = consts.tile([128, HT, PT], BF16, tag="WhhT")
            nc.vector.memset(WhhT[:], 0.0)
            WxxT = consts.tile([128, HT, PT], BF16, tag="WxxT")
            nc.vector.memset(WxxT[:], 0.0)
            tp_ap = tp[:]
            for wt, slot0 in ((WhhT, 0), (WxxT, HT * 50)):
                wt_ap = wt[:]
                nc.vector.tensor_copy(
                    _ap(wt_ap, 0, [wt_ap.ap[0], [PT, HT], [1, R]]),
                    _ap(tp_ap, slot0, [tp_ap.ap[0], [50, HT], [1, R]]),
                )
                nc.vector.tensor_copy(
                    _ap(wt_ap, PB, [wt_ap.ap[0], [PT, HT], [1, R]]),
                    _ap(tp_ap, slot0, [tp_ap.ap[0], [50, HT], [1, R]]),
                )

            prm = consts.tile([128, 16], F32, tag="prm")
            nc.scalar.dma_start(out=prm[:], in_=prm_d)
            prmsum = consts.tile([128, 16], F32, tag="prmsum")
            nc.vector.tensor_copy(prmsum[:], prm[:])

            # ---- Wa as a column vector [49, 1] f32 ----
            WaT = consts.tile([R, 1], F32)
            nc.sync.dma_start(out=WaT[:], in_=_ap(Wa_d, 0, [[1, R], [1, 1]]))

            # ---- per-partition scale/bias const vectors for the 3 groups:
            # rows 0:49 -> params[2g], rows 49:98 -> params[2g+1] ----
            def param_vec(tag, vals):
                vecs = []
                for g in range(NG):
                    v = consts.tile([128, 1], F32, tag=f"{tag}{g}")
                    nc.vector.memset(v[32:PB, :], 0.0)
                    nc.vector.memset(v[96:128, :], 0.0)
                    nc.vector.memset(v[0:R, :], float(vals[2 * g]))
                    nc.vector.memset(v[PB:PT, :], float(vals[2 * g + 1]))
                    vecs.append(v)
                return vecs

            gA = param_vec("gA", FG)
            dA = param_vec("dA", FD)
            gB = param_vec("gB", FGP)
            dB = param_vec("dB", FDP)
            # cwa3[p, g]: rows 0:49 = FC[2g]*Wa, rows 49:98 = FC[2g+1]*Wa
            cwa3 = consts.tile([128, NG], F32, tag="cwa3")
            nc.vector.memset(cwa3[32:PB, :], 0.0)
            nc.vector.memset(cwa3[96:128, :], 0.0)
            for g in range(NG):
                nc.vector.tensor_scalar_mul(
                    cwa3[0:R, g : g + 1], WaT[:], float(FC[2 * g])
                )
                nc.vector.tensor_scalar_mul(
                    cwa3[PB:PT, g : g + 1], WaT[:], float(FC[2 * g + 1])
                )
            # ones column for the softmax denominator matmul (both blocks:
            # bb=0 reads rows 0:49, bb=1 reads rows 64:113 to match xb's
            # base partition)
            onesb = consts.tile([128, 1], BF16, tag="onesb")
            nc.vector.memset(onesb[0:R, :], 1.0)
            nc.vector.memset(onesb[PB : PB + R, :], 1.0)

            # ---- per-pair state (tile versions) ----
            st = [dict() for _ in range(NP)]

            def S_load(p):
                b0 = 2 * p
                xb = xbp.tile([128, H], BF16, tag="xb")
                nc.scalar.dma_start(
                    out=xb[:], in_=_ap(X_d, b0 * KP * H, [[H, 2 * KP], [1, H]])
                )
                hxT = hxT_pool.tile([128, HT, 354], BF16, tag="hxT")
                nc.sync.dma_start(
                    out=hxT[:],
                    in_=_ap(
                        hx_d, p * 128 * HT * 354, [[HT * 354, 128], [1, HT * 354]]
                    ),
                )
                st[p]["xb"] = xb
                st[p]["hxT"] = hxT

            def S_mm1(p):
                cc = pcc.tile([PT, 354], F32, tag="cc")
                hxT = st[p]["hxT"]
                for j in range(HT):
                    nc.tensor.matmul(
                        cc[:, 0:256],
                        WhhT[:, j, :],
                        hxT[:, j, 0:256],
                        start=(j == 0),
                        stop=(j == HT - 1),
                    )
                for j in range(HT):
                    nc.tensor.matmul(
                        cc[:, 256:354],
                        WxxT[:, j, :],
                        hxT[:, j, 256:354],
                        start=(j == 0),
                        stop=(j == HT - 1),
                    )
                st[p]["cc"] = cc

            def S_repl_big(p):
                # single full-width PSUM->SBUF copy (doubled rows already)
                ccr = ccrp.tile([128, 354], F32, tag="ccr")
                cc = st[p]["cc"]
                nc.gpsimd.memset(ccr[32:PB, :], 0.0)
                nc.vector.tensor_copy(ccr[0:PB, :], cc[0:PB, :])
                nc.scalar.copy(ccr[PB:PT, :], cc[PB:PT, :])
                st[p]["ccr"] = ccr

            def S_repl_small(p):
                pass

            def S_tanh(p):
                ccr = st[p]["ccr"]
                SA = SAp.tile([128, NG, 256], BF16, tag="SA")
                for g in range(NG):
                    nc.scalar.activation(
                        SA[0:PT, g, :],
                        ccr[0:PT, 0:256],
                        mybir.ActivationFunctionType.Tanh,
                        bias=dA[g][0:PT, :],
                        scale=gA[g][0:PT, :],
                    )
                sbt = sbtp.tile([128, NG, 2 * R], BF16, tag="sbt")
                for g in range(NG):
                    nc.scalar.activation(
                        sbt[0:PT, g, :],
                        ccr[0:PT, 256:354],
                        mybir.ActivationFunctionType.Tanh,
                        bias=dB[g][0:PT, :],
                        scale=gB[g][0:PT, :],
                    )
                st[p]["SA"] = SA
                st[p]["sbt"] = sbt

            def S_fold(p):
                sbf = sbfp.tile([128, NG, 2 * R], BF16, tag="sbf")
                c3 = cwa3[:]
                nc.gpsimd.tensor_tensor(
                    sbf[0:PT, :, :],
                    st[p]["sbt"][0:PT, :, :],
                    _ap(c3, 0, [[c3.ap[0][0], PT], [1, NG], [0, 2 * R]]),
                    mybir.AluOpType.mult,
                )
                st[p]["sbf"] = sbf

            def S_zT(p):
                zps = psZ.tile([128, 2, 129], F32, tag="z")
                SA = st[p]["SA"]
                sbf = st[p]["sbf"]
                for bb in range(2):
                    for g in range(NG):
                        nc.tensor.matmul(
                            zps[0:R, bb, 0:128],
                            sbf[0:PT, g, bb * R : (bb + 1) * R],
                            SA[0:PT, g, bb * 128 : (bb + 1) * 128],
                            start=(g == 0),
                            stop=(g == NG - 1),
                        )
                st[p]["zps"] = zps

            def S_exp(p):
                ez = ezp.tile([128, 2, 128], BF16, tag="ez")
                nc.scalar.activation(
                    ez[0:R, :, :],
                    st[p]["zps"][0:R, :, 0:128],
                    mybir.ActivationFunctionType.Exp,
                )
                # replicate for bb=1's matmuls (xb block at base partition 64)
                nc.gpsimd.tensor_copy(ez[PB : PB + R, 1, :], ez[0:R, 1, :])
                st[p]["ez"] = ez

            def S_den_mm3(p):
                zps = st[p]["zps"]
                ez = st[p]["ez"]
                xb = st[p]["xb"]
                for bb in range(2):
                    pb = KP * bb
                    nc.tensor.matmul(
                        zps[:, bb, 128:129],
                        ez[pb : pb + R, bb, :],
                        onesb[pb : pb + R, :],
                        start=True,
                        stop=True,
                    )
                rden = rdnp.tile([128, 2], F32, tag="rden")
                nc.vector.reciprocal(rden[:], zps[:, :, 128:129])
                rd = rden[:]
                osb = osbp.tile([128, 2, H], BF16, tag="osb")
                for bb in range(2):
                    pb = KP * bb
                    for hh in range(2):
                        ob = psO.tile([128, 512], F32, tag="ob")
                        nc.tensor.matmul(
                            ob[:],
                            ez[pb : pb + R, bb, :],
                            xb[pb : pb + R, hh * 512 : (hh + 1) * 512],
                            start=True,
                            stop=True,
                        )
                        nc.vector.tensor_tensor(
                            osb[:, bb, hh * 512 : (hh + 1) * 512],
                            ob[:],
                            _ap(rd, bb, [rd.ap[0], [0, 512]]),
                            mybir.AluOpType.mult,
                        )
                st[p]["osb"] = osb

            def S_wr(p):
                nc.scalar.dma_start(
                    out=_ap(out_d, 2 * p * T * H, [[H, T], [T * H, 2], [1, H]]),
                    in_=st[p]["osb"][:],
                )

            # ---- 6-deep software pipeline ----
            for i in range(NP + 5):
                if i >= 5:
                    S_wr(i - 5)
                if 3 <= i < NP + 3:
                    S_zT(i - 3)
                if 4 <= i < NP + 4:
                    S_exp(i - 4)
                    S_den_mm3(i - 4)
                if i < NP:
                    S_load(i)
                if 1 <= i < NP + 1:
                    S_mm1(i - 1)
                if 2 <= i < NP + 2:
                    S_tanh(i - 2)
                    S_fold(i - 2)
                if 1 <= i < NP + 1:
                    S_repl_big(i - 1)
                    S_repl_small(i - 1)

    nc.compile()
    return nc


def _get_nc():
    if "nc" not in _CACHE:
        _CACHE["nc"] = build()
    return _CACHE["nc"]


def make_in_maps(X, h_t, Wx, Wh, Wa):
    import ml_dtypes

    Xf = np.asarray(X, dtype=np.float32)
    Xc = Xf.astype(ml_dtypes.bfloat16)
    Xb = np.zeros((B, KP, H), dtype=ml_dtypes.bfloat16)
    Xb[:, :K, :] = Xc
    hc = np.asarray(h_t, dtype=np.float32).astype(ml_dtypes.bfloat16)
    # pack hxT[pair, p, j, :]: cols 0:256 = h^T (u=bb*128+t), 256:354 = X^T
    NPAIR = B // 2
    hx = np.empty((NPAIR, 128, HT, 354), dtype=ml_dtypes.bfloat16)
    # h part: [pair, bb, t, j, p] -> [pair, p, j, bb*128+t]
    hx[:, :, :, 0:256] = (
        hc.reshape(NPAIR, 2, T, HT, 128)
        .transpose(0, 4, 3, 1, 2)
        .reshape(NPAIR, 128, HT, 256)
    )
    hx[:, :, :, 256:354] = (
        Xc.reshape(NPAIR, 2, K, HT, 128)
        .transpose(0, 4, 3, 1, 2)
        .reshape(NPAIR, 128, HT, 98)
    )
    Wx = np.ascontiguousarray(Wx, dtype=np.float32)
    Wh = np.ascontiguousarray(Wh, dtype=np.float32)
    Wa = np.ascontiguousarray(Wa, dtype=np.float32)
    in_maps = [
        {
            "X": Xb[c * BL : (c + 1) * BL],
            "hxT": np.ascontiguousarray(hx[c * NP : (c + 1) * NP]),
            "Wx": Wx,
            "Wh": Wh,
            "Wa": Wa,
            "prm": np.zeros((128, 16), np.float32),
        }
        for c in range(NCORES)
    ]
    return in_maps


def kernel(X, h_t, Wx, Wh, Wa):
    nc = _get_nc()
    in_maps = make_in_maps(X, h_t, Wx, Wh, Wa)
    last_err = None
    for _attempt in range(3):
        try:
            res = run_bass_kernel_spmd(nc, in_maps, core_ids=list(range(NCORES)))
            break
        except Exception as e:  # transient INTERNAL device errors: retry
            last_err = e
    else:
        raise last_err
    return np.concatenate(
        [np.asarray(res.results[c]["out"]).astype(np.float32) for c in range(NCORES)],
        axis=0,
    )


# revision 6
# speedup vs baseline: 1.1214x; 1.0090x over previous
"""Trainium2 Bass kernel for nn_AttentionBlock: 8-core data-parallel over batch.

Reference computation (per batch b):
  cx = X[b] @ Wx^T               [K,R]   (K=49 regions, R=49, H=1024)
  ch = h_t[b] @ Wh^T             [T,R]   (T=128)
  z[t,k] = sum_r Wa[r] * tanh(cx[k,r] + ch[t,r])
  alpha = softmax_k(z)           [T,K]
  out[b] = alpha @ X[b]          [T,H]

Design (measured ~66us median vs the 148846 ns v3 baseline):
  - host pre-packs the DMA-heavy layouts: hxT[pair, p, j, :] holds bf16 h^T
    (cols 0:256, u=bb*128+t) and X^T (cols 256:354, q=bb*49+k) already in
    matmul-rhs layout -> one large-granule DMA per pair, zero device-side
    transposes or casts; X padded to 64 rows bf16 (partition-base-64 legal)
  - inputs fed bf16, output written bf16 and upcast on host: HBM traffic
    ~12.5MB/core vs 20.4MB all-f32
  - ch/cx via DOUBLED stationaries [Wh|0|Wh], [Wx|0|Wx] built on device:
    cc[113, 0:256]=ch at rows 0:49 AND 64:113, cc[113, 256:354]=cx same;
    gap rows 49:64 are zeros (zero stationary cols)
  - rank-6 tanh-product fit tanh(a+b) ~= sum_m c_m T(g_m a+d_m)T(g'_m b+d'_m),
    m-pairs packed at partition blocks 0:49 / 64:113, 3 ACT ops per side with
    per-partition scale/bias; z as 3 PSUM-accumulated matmuls per batch
  - z computed TRANSPOSED zT[k, (bb,t)]: no alpha transpose, no
    max-subtraction (|z| <= ~6 analytically); denominator via a 1-column
    ones-matmul; 1/den folded into the PSUM->SBUF output drain on DVE
  - 6-deep software pipeline; per-iteration emission only references work
    >= 1 iteration old per engine (no head-of-line blocking); PE ramps to
    its 2.4GHz P-state once streams are dense (~3us continuous)
  - pitfalls baked in (each cost a debugging session):
    * engine ops need partition bases that are multiples of 32
    * gpsimd (Pool) cannot touch PSUM; bf16 memset into PSUM is illegal
    * the gpsimd memset in S_repl looks redundant but removing it makes the
      NEFF fail at runtime (scheduling artifact); odd-inner-dim inputs like
      [128, 15] f32 also crash -- keep ExternalInputs 16/128-padded
    * concurrent XBAR dma_start_transpose on both HWDGE rings corrupt each
      other (not used here, but measured)
  - kernel() retries run_bass_kernel_spmd up to 3x: the axon device path
    intermittently fails with INTERNAL errors unrelated to the kernel
"""

import sys

sys.path.insert(0, "/opt/trn_rl_repo")

import numpy as np

import concourse.bass as bass
import concourse.bacc as bacc
import concourse.tile as tile
from concourse import mybir
from concourse.bass_utils import run_bass_kernel_spmd
from concourse.masks import make_identity

B, T, K, H = 128, 128, 49, 1024
R = 49
NCORES = 8
BL = B // NCORES  # batches per core
NP = BL // 2  # pairs per core
HT = H // 128  # h tiles
KP = 64  # padded K rows for X (host-side zero pad)
PB = 64  # partition base of the second (m, r) block (must be mult of 32)
PT = PB + R  # 113 partitions used
F32 = mybir.dt.float32
BF16 = mybir.dt.bfloat16

# rank-6 tanh-product fit (LAM=0.03 gauss-weighted, sigma=0.64, A=3.2):
# tanh(a+b) ~= sum_m FC[m] * tanh(FG[m]*a + FD[m]) * tanh(FGP[m]*b + FDP[m])
# rank-4 refit (end-to-end 6.2e-3 incl bf16 vs rank-6's 3.7e-3; gate 2e-2)
FG = [0.9794, -0.9901, 1.0111, -0.0645]
FD = [-0.1872, 0.0508, 3.1349, 0.0117]
FGP = [0.9891, -0.9785, -0.0725, -1.013]
FDP = [0.0506, -0.1871, -0.0132, 3.1413]
FC = [-5.0578, 5.0574, -3.2168, -3.6059]
NG = 2  # number of (m-pair) partition groups

_CACHE = {}


def _ap(base, off, dims):
    """Custom access pattern on the tensor underlying `base` (an AP)."""
    return bass.AP(tensor=base.tensor, offset=base.offset + off, ap=dims)


def build():
    nc = bacc.Bacc("TRN2", target_bir_lowering=False, debug=False, num_devices=NCORES)

    X_d = nc.dram_tensor("X", [BL, KP, H], BF16, kind="ExternalInput").ap()
    # host-packed transposed rhs tiles: hxT[pair, p, j, 0:256] = h^T block
    # (u = bb*128+t), cols 256:354 = X^T block (q = bb*49+k)
    hx_d = nc.dram_tensor("hxT", [NP, 128, HT, 354], BF16, kind="ExternalInput").ap()
    Wx_d = nc.dram_tensor("Wx", [R, H], F32, kind="ExternalInput").ap()
    Wh_d = nc.dram_tensor("Wh", [R, H], F32, kind="ExternalInput").ap()
    Wa_d = nc.dram_tensor("Wa", [1, R], F32, kind="ExternalInput").ap()
    prm_d = nc.dram_tensor("prm", [128, 16], F32, kind="ExternalInput").ap()
    out_d = nc.dram_tensor("out", [BL, T, H], BF16, kind="ExternalOutput").ap()

    with tile.TileContext(nc) as tc:
        with (
            tc.tile_pool(name="consts", bufs=1) as consts,
            tc.tile_pool(name="hxTp", bufs=3) as hxT_pool,
            tc.tile_pool(name="xbp", bufs=5) as xbp,
            tc.tile_pool(name="ccrp", bufs=3) as ccrp,
            tc.tile_pool(name="SAp", bufs=3) as SAp,
            tc.tile_pool(name="sbtp", bufs=2) as sbtp,
            tc.tile_pool(name="sbfp", bufs=3) as sbfp,
            tc.tile_pool(name="ezp", bufs=3) as ezp,
            tc.tile_pool(name="rdnp", bufs=3) as rdnp,
            tc.tile_pool(name="osbp", bufs=3) as osbp,
            tc.tile_pool(name="ptp", bufs=1, space="PSUM") as ptp,
            tc.tile_pool(name="pcc", bufs=2, space="PSUM") as pcc,
            tc.tile_pool(name="psZ", bufs=2, space="PSUM") as psZ,
            tc.tile_pool(name="psO", bufs=3, space="PSUM") as psO,
        ):
            # ---- identity for weight PE transposes ----
            identb = consts.tile([128, 128], BF16)
            make_identity(nc, identb[:])

            # ---- weights: load f32, cast bf16, PE-transpose into combined
            # stationary WhxT[128, j, 98]: cols 0:49 = WhT_j, 49:98 = WxT_j ----
            wnh = consts.tile([R, H], F32, tag="wnh")
            nc.sync.dma_start(out=wnh[:], in_=Wh_d)
            wnx = consts.tile([R, H], F32, tag="wnx")
            nc.sync.dma_start(out=wnx[:], in_=Wx_d)
            wbh = consts.tile([R, H], BF16, tag="wbh")
            nc.vector.tensor_copy(wbh[:], wnh[:])
            wbx = consts.tile([R, H], BF16, tag="wbx")
            nc.vector.tensor_copy(wbx[:], wnx[:])
            tp = ptp.tile([128, 800], BF16, tag="tp")
            for j in range(HT):
                nc.tensor.transpose(
                    tp[:, j * 50 : j * 50 + R],
                    wbh[:, j * 128 : (j + 1) * 128],
                    identb[0:R, 0:R],
                )
                nc.tensor.transpose(
                    tp[:, (HT + j) * 50 : (HT + j) * 50 + R],
                    wbx[:, j * 128 : (j + 1) * 128],
                    identb[0:R, 0:R],
                )
            WhhT = consts.tile([128, HT, PT], BF16, tag="WhhT")
            nc.vector.memset(WhhT[:], 0.0)
            WxxT = consts.tile([128, HT, PT], BF16, tag="WxxT")
            nc.vector.memset(WxxT[:], 0.0)
            tp_ap = tp[:]
            for wt, slot0 in ((WhhT, 0), (WxxT, HT * 50)):
                wt_ap = wt[:]
                nc.vector.tensor_copy(
                    _ap(wt_ap, 0, [wt_ap.ap[0], [PT, HT], [1, R]]),
                    _ap(tp_ap, slot0, [tp_ap.ap[0], [50, HT], [1, R]]),
                )
                nc.vector.tensor_copy(
                    _ap(wt_ap, PB, [wt_ap.ap[0], [PT, HT], [1, R]]),
                    _ap(tp_ap, slot0, [tp_ap.ap[0], [50, HT], [1, R]]),
                )

            prm = consts.tile([128, 16], F32, tag="prm")
            nc.scalar.dma_start(out=prm[:], in_=prm_d)
            prmsum = consts.tile([128, 16], F32, tag="prmsum")
            nc.vector.tensor_copy(prmsum[:], prm[:])

            # ---- Wa as a column vector [49, 1] f32 ----
            WaT = consts.tile([R, 1], F32)
            nc.sync.dma_start(out=WaT[:], in_=_ap(Wa_d, 0, [[1, R], [1, 1]]))

            # ---- per-partition scale/bias const vectors for the 3 groups:
            # rows 0:49 -> params[2g], rows 49:98 -> params[2g+1] ----
            def param_vec(tag, vals):
                vecs = []
                for g in range(NG):
                    v = consts.tile([128, 1], F32, tag=f"{tag}{g}")
                    nc.gpsimd.memset(v[32:PB, :], 0.0)
                    nc.gpsimd.memset(v[96:128, :], 0.0)
                    nc.gpsimd.memset(v[0:R, :], float(vals[2 * g]))
                    nc.gpsimd.memset(v[PB:PT, :], float(vals[2 * g + 1]))
                    vecs.append(v)
                return vecs

            gA = param_vec("gA", FG)
            dA = param_vec("dA", FD)
            gB = param_vec("gB", FGP)
            dB = param_vec("dB", FDP)
            # cwa3[p, g]: rows 0:49 = FC[2g]*Wa, rows 49:98 = FC[2g+1]*Wa
            cwa3 = consts.tile([128, NG], F32, tag="cwa3")
            nc.gpsimd.memset(cwa3[32:PB, :], 0.0)
            nc.gpsimd.memset(cwa3[96:128, :], 0.0)
            for g in range(NG):
                nc.vector.tensor_scalar_mul(
                    cwa3[0:R, g : g + 1], WaT[:], float(FC[2 * g])
                )
                nc.vector.tensor_scalar_mul(
                    cwa3[PB:PT, g : g + 1], WaT[:], float(FC[2 * g + 1])
                )
            # ones column for the softmax denominator matmul (both blocks:
            # bb=0 reads rows 0:49, bb=1 reads rows 64:113 to match xb's
            # base partition)
            onesb = consts.tile([128, 1], BF16, tag="onesb")
            nc.gpsimd.memset(onesb[0:R, :], 1.0)
            nc.gpsimd.memset(onesb[PB : PB + R, :], 1.0)

            # ---- per-pair state (tile versions) ----
            st = [dict() for _ in range(NP)]

            def S_load(p):
                b0 = 2 * p
                xb = xbp.tile([128, H], BF16, tag="xb")
                nc.scalar.dma_start(
                    out=xb[:], in_=_ap(X_d, b0 * KP * H, [[H, 2 * KP], [1, H]])
                )
                hxT = hxT_pool.tile([128, HT, 354], BF16, tag="hxT")
                nc.sync.dma_start(
                    out=hxT[:],
                    in_=_ap(
                        hx_d, p * 128 * HT * 354, [[HT * 354, 128], [1, HT * 354]]
                    ),
                )
                st[p]["xb"] = xb
                st[p]["hxT"] = hxT

            def S_mm1(p):
                cc = pcc.tile([PT, 354], F32, tag="cc")
                hxT = st[p]["hxT"]
                for j in range(HT):
                    nc.tensor.matmul(
                        cc[:, 0:256],
                        WhhT[:, j, :],
                        hxT[:, j, 0:256],
                        start=(j == 0),
                        stop=(j == HT - 1),
                    )
                for j in range(HT):
                    nc.tensor.matmul(
                        cc[:, 256:354],
                        WxxT[:, j, :],
                        hxT[:, j, 256:354],
                        start=(j == 0),
                        stop=(j == HT - 1),
                    )
                st[p]["cc"] = cc

            def S_repl_big(p):
                # single full-width PSUM->SBUF copy (doubled rows already)
                ccr = ccrp.tile([128, 354], F32, tag="ccr")
                cc = st[p]["cc"]
                nc.gpsimd.memset(ccr[32:PB, :], 0.0)
                nc.vector.tensor_copy(ccr[0:PB, :], cc[0:PB, :])
                nc.scalar.copy(ccr[PB:PT, :], cc[PB:PT, :])
                st[p]["ccr"] = ccr

            def S_repl_small(p):
                pass

            def S_tanh(p):
                ccr = st[p]["ccr"]
                SA = SAp.tile([128, NG, 256], BF16, tag="SA")
                for g in range(NG):
                    nc.scalar.activation(
                        SA[0:PT, g, :],
                        ccr[0:PT, 0:256],
                        mybir.ActivationFunctionType.Tanh,
                        bias=dA[g][0:PT, :],
                        scale=gA[g][0:PT, :],
                    )
                sbt = sbtp.tile([128, NG, 2 * R], BF16, tag="sbt")
                for g in range(NG):
                    nc.scalar.activation(
                        sbt[0:PT, g, :],
                        ccr[0:PT, 256:354],
                        mybir.ActivationFunctionType.Tanh,
                        bias=dB[g][0:PT, :],
                        scale=gB[g][0:PT, :],
                    )
                st[p]["SA"] = SA
                st[p]["sbt"] = sbt

            def S_fold(p):
                sbf = sbfp.tile([128, NG, 2 * R], BF16, tag="sbf")
                c3 = cwa3[:]
                nc.gpsimd.tensor_tensor(
                    sbf[0:PT, :, :],
                    st[p]["sbt"][0:PT, :, :],
                    _ap(c3, 0, [[c3.ap[0][0], PT], [1, NG], [0, 2 * R]]),
                    mybir.AluOpType.mult,
                )
                st[p]["sbf"] = sbf

            def S_zT(p):
                zps = psZ.tile([128, 2, 129], F32, tag="z")
                SA = st[p]["SA"]
                sbf = st[p]["sbf"]
                for bb in range(2):
                    for g in range(NG):
                        nc.tensor.matmul(
                            zps[0:R, bb, 0:128],
                            sbf[0:PT, g, bb * R : (bb + 1) * R],
                            SA[0:PT, g, bb * 128 : (bb + 1) * 128],
                            start=(g == 0),
                            stop=(g == NG - 1),
                        )
                st[p]["zps"] = zps

            def S_exp(p):
                ez = ezp.tile([128, 2, 128], BF16, tag="ez")
                nc.scalar.activation(
                    ez[0:R, :, :],
                    st[p]["zps"][0:R, :, 0:128],
                    mybir.ActivationFunctionType.Exp,
                )
                # replicate for bb=1's matmuls (xb block at base partition 64)
                nc.gpsimd.tensor_copy(ez[PB : PB + R, 1, :], ez[0:R, 1, :])
                st[p]["ez"] = ez

            def S_den_mm3(p):
                zps = st[p]["zps"]
                ez = st[p]["ez"]
                xb = st[p]["xb"]
                for bb in range(2):
                    pb = KP * bb
                    nc.tensor.matmul(
                        zps[:, bb, 128:129],
                        ez[pb : pb + R, bb, :],
                        onesb[pb : pb + R, :],
                        start=True,
                        stop=True,
                    )
                rden = rdnp.tile([128, 2], F32, tag="rden")
                nc.vector.reciprocal(rden[:], zps[:, :, 128:129])
                rd = rden[:]
                osb = osbp.tile([128, 2, H], BF16, tag="osb")
                for bb in range(2):
                    pb = KP * bb
                    for hh in range(2):
                        ob = psO.tile([128, 512], F32, tag="ob")
                        nc.tensor.matmul(
                            ob[:],
                            ez[pb : pb + R, bb, :],
                            xb[pb : pb + R, hh * 512 : (hh + 1) * 512],
                            start=True,
                            stop=True,
                        )
                        nc.vector.tensor_tensor(
                            osb[:, bb, hh * 512 : (hh + 1) * 512],
                            ob[:],
                            _ap(rd, bb, [rd.ap[0], [0, 512]]),
                            mybir.AluOpType.mult,
                        )
                st[p]["osb"] = osb

            def S_wr(p):
                nc.scalar.dma_start(
                    out=_ap(out_d, 2 * p * T * H, [[H, T], [T * H, 2], [1, H]]),
                    in_=st[p]["osb"][:],
                )

            # ---- 6-deep software pipeline ----
            for i in range(NP + 5):
                if i >= 5:
                    S_wr(i - 5)
                if 3 <= i < NP + 3:
                    S_zT(i - 3)
                if 4 <= i < NP + 4:
                    S_exp(i - 4)
                    S_den_mm3(i - 4)
                if i < NP:
                    S_load(i)
                if 1 <= i < NP + 1:
                    S_mm1(i - 1)
                if 2 <= i < NP + 2:
                    S_tanh(i - 2)
                    S_fold(i - 2)
                if 1 <= i < NP + 1:
                    S_repl_big(i - 1)
                    S_repl_small(i - 1)

    nc.compile()
    return nc


def _get_nc():
    if "nc" not in _CACHE:
        _CACHE["nc"] = build()
    return _CACHE["nc"]


def make_in_maps(X, h_t, Wx, Wh, Wa):
    import ml_dtypes

    Xf = np.asarray(X, dtype=np.float32)
    Xc = Xf.astype(ml_dtypes.bfloat16)
    Xb = np.zeros((B, KP, H), dtype=ml_dtypes.bfloat16)
    Xb[:, :K, :] = Xc
    hc = np.asarray(h_t, dtype=np.float32).astype(ml_dtypes.bfloat16)
    # pack hxT[pair, p, j, :]: cols 0:256 = h^T (u=bb*128+t), 256:354 = X^T
    NPAIR = B // 2
    hx = np.empty((NPAIR, 128, HT, 354), dtype=ml_dtypes.bfloat16)
    # h part: [pair, bb, t, j, p] -> [pair, p, j, bb*128+t]
    hx[:, :, :, 0:256] = (
        hc.reshape(NPAIR, 2, T, HT, 128)
        .transpose(0, 4, 3, 1, 2)
        .reshape(NPAIR, 128, HT, 256)
    )
    hx[:, :, :, 256:354] = (
        Xc.reshape(NPAIR, 2, K, HT, 128)
        .transpose(0, 4, 3, 1, 2)
        .reshape(NPAIR, 128, HT, 98)
    )
    Wx = np.ascontiguousarray(Wx, dtype=np.float32)
    Wh = np.ascontiguousarray(Wh, dtype=np.float32)
    Wa = np.ascontiguousarray(Wa, dtype=np.float32)
    in_maps = [
        {
            "X": Xb[c * BL : (c + 1) * BL],
            "hxT": np.ascontiguousarray(hx[c * NP : (c + 1) * NP]),
            "Wx": Wx,
            "Wh": Wh,
            "Wa": Wa,
            "prm": np.zeros((128, 16), np.float32),
        }
        for c in range(NCORES)
    ]
    return in_maps


def kernel(X, h_t, Wx, Wh, Wa):
    nc = _get_nc()
    in_maps = make_in_maps(X, h_t, Wx, Wh, Wa)
    last_err = None
    for _attempt in range(3):
        try:
            res = run_bass_kernel_spmd(nc, in_maps, core_ids=list(range(NCORES)))
            break
        except Exception as e:  # transient INTERNAL device errors: retry
            last_err = e
    else:
        raise last_err
    return np.concatenate(
        [np.asarray(res.results[c]["out"]).astype(np.float32) for c in range(NCORES)],
        axis=0,
    )
